# revision 11
# baseline (speedup 1.0000x reference)
"""Trainium2 Bass kernel for nn_CGNN (3-layer GNN message passing).

Math per layer:  prop = A @ h  (A sparse COO: out[row] += C * h[col]);
z = prop @ W + b; if not last: h' = l2norm_rows(relu(z)).

Distribution: destination-node sharding across 8 cores (6272 rows each, 49
tiles of 128).  Each core gathers source rows h[col] for its edges with the
custom SWDGE dma_gather (4 queues), and performs the segment-sum as PE
matmuls:  propT[f, d] = sum_e G[e, f] * S[e, d]  (S built on host with C
folded in, bf16, edges grouped per dest tile, deduped by source; group sizes
uniform across cores so one SPMD program serves all 8 cores).

Nodes are renumbered into two regions: A = tiles 0..24 of every core
(25600 rows), B = tiles 25..48 (24576 rows) - both < 32767 so gather
indices fit int16.  h is AllGathered between layers in two halves (AG1 for
region A fired after the first 25 tiles finish, AG2 after the rest), and
each layer runs in two phases (region-A edges accumulated into an SBUF
propT buffer, then region-B edges + Linear + norm), so the next layer's
region-A gathers overlap the tail of the current layer and AG2.

Self-contained: hardcodes all shapes from the problem spec.
"""
import os

import numpy as np
import ml_dtypes

# ---------------------------------------------------------------- constants
N = 50000
E = 800000
D = 128
NCLS = 64
NCORES = 8
P = 128
PAD_N = 50176            # 8 * 6272
SHARD = PAD_N // NCORES  # 6272
NT = SHARD // P          # 49 dest tiles per core
NT_A = (NT + 1) // 2     # 25 tiles -> region A
NT_B = NT - NT_A
REG_A = NCORES * NT_A * P
REG_B = NCORES * NT_B * P
BATCH_CH = 16            # chunks per gather call -> 2048 idx
BATCH = BATCH_CH * P
NQ = 4                   # SWDGE queues
EPS = 1e-12

bf16 = ml_dtypes.bfloat16


def _refresh():
    """Recompute derived constants after monkeypatching (debug shrink)."""
    global NT_A, NT_B, REG_A, REG_B, BATCH
    NT_A = (NT + 1) // 2
    NT_B = NT - NT_A
    REG_A = NCORES * NT_A * P
    REG_B = NCORES * NT_B * P
    BATCH = BATCH_CH * P


def _region_pos(node):
    """Map padded-global node id -> (region h, index within region)."""
    c = node // SHARD
    r = node % SHARD
    t = r // P
    h = (t >= NT_A).astype(np.int64)
    pa = c * NT_A * P + r
    pb = c * NT_B * P + (r - NT_A * P)
    return h, np.where(h == 0, pa, pb)


def _wrap_idxs(idx):
    """[L] -> [128, L/16] int16 wrapped (pos i = s*16 + p), replicated x8."""
    n = idx.shape[0]
    assert n % 16 == 0
    w = idx.astype(np.int16).reshape(n // 16, 16).T
    return np.ascontiguousarray(np.tile(w, (8, 1)))


# ---------------------------------------------------------------- host prep
def _prepare(edge_index, C_vals):
    row = np.asarray(edge_index[0], dtype=np.int64)
    col = np.asarray(edge_index[1], dtype=np.int64)
    C = np.asarray(C_vals, dtype=np.float32)

    core = row // SHARD
    tile_of = (row % SHARD) // P
    dloc = row % P
    half, ridx = _region_pos(col)

    ucount = np.zeros((NCORES, NT, 2), np.int64)
    groups = {}
    key = ((core * NT + tile_of) * 2 + half)
    order = np.argsort(key, kind="stable")
    ks = key[order]
    bounds = np.searchsorted(ks, np.arange(NCORES * NT * 2 + 1))
    for c in range(NCORES):
        for t in range(NT):
            for h in (0, 1):
                k = (c * NT + t) * 2 + h
                sel = order[bounds[k]:bounds[k + 1]]
                u, inv = np.unique(ridx[sel], return_inverse=True)
                groups[(c, t, h)] = (u, inv, dloc[sel], C[sel])
                ucount[c, t, h] = len(u)

    gsz = ((ucount.max(axis=0) + P - 1) // P) * P  # [NT, 2]
    nl = (gsz[:, 0] // P).astype(np.int64)
    nh = (gsz[:, 1] // P).astype(np.int64)
    lo_off = np.concatenate([[0], np.cumsum(nl)])[:NT]
    hi_off = np.concatenate([[0], np.cumsum(nh)])[:NT]
    llo_ch = int(nl.sum())
    lhi_ch = int(nh.sum())
    nchunk = llo_ch + lhi_ch
    nbl = -(-llo_ch // BATCH_CH)
    nbh = -(-lhi_ch // BATCH_CH)

    idx_lo_all, idx_hi_all, s_all = [], [], []
    for c in range(NCORES):
        s_mat = np.zeros((P, nchunk, P), np.float32)
        str_lo = np.zeros(nbl * BATCH, np.int64)
        str_hi = np.zeros(nbh * BATCH, np.int64)
        for t in range(NT):
            for h in (0, 1):
                u, inv, dl, cv = groups[(c, t, h)]
                base_s = (lo_off[t] if h == 0 else llo_ch + hi_off[t]) * P
                r = base_s + inv
                np.add.at(s_mat, (r % P, r // P, dl), cv)
                stream = str_lo if h == 0 else str_hi
                boff = (lo_off[t] if h == 0 else hi_off[t]) * P
                stream[boff:boff + len(u)] = u
        s_all.append(s_mat.astype(bf16))
        idx_lo_all.append(_wrap_idxs(str_lo))
        idx_hi_all.append(_wrap_idxs(str_hi))

    return {
        "nl": nl, "nh": nh, "nchunk": nchunk, "nbl": nbl, "nbh": nbh,
        "llo_ch": llo_ch, "lo_off": lo_off, "hi_off": hi_off,
        "idx_lo": idx_lo_all, "idx_hi": idx_hi_all, "s_mat": s_all,
    }


# ---------------------------------------------------------------- device
def _build(sched):
    import concourse.bacc as bacc
    import concourse.mybir as mybir
    import concourse.tile as tile
    from concourse import library_config

    nl, nh = sched["nl"], sched["nh"]
    nchunk, nbl, nbh = sched["nchunk"], sched["nbl"], sched["nbh"]
    llo_ch = sched["llo_ch"]
    lo_off, hi_off = sched["lo_off"], sched["hi_off"]
    nsb = -(-nchunk // BATCH_CH)

    f32 = mybir.dt.float32
    b16 = mybir.dt.bfloat16

    nc = bacc.Bacc("TRN2", num_devices=NCORES, num_swdge_queues=NQ)
    xbf = nc.dram_tensor("xbf", [PAD_N, D], b16, kind="ExternalInput")
    s_in = nc.dram_tensor("s_mat", [P, nchunk, P], b16, kind="ExternalInput")
    ilo = nc.dram_tensor("idx_lo", [P, nbl * BATCH // 16], mybir.dt.int16,
                         kind="ExternalInput")
    ihi = nc.dram_tensor("idx_hi", [P, nbh * BATCH // 16], mybir.dt.int16,
                         kind="ExternalInput")
    w_in = [nc.dram_tensor(f"W{i+1}", [D, D if i < 2 else NCLS], b16,
                           kind="ExternalInput") for i in range(3)]
    b_in = [nc.dram_tensor(f"b{i+1}", [1, D if i < 2 else NCLS], b16,
                           kind="ExternalInput") for i in range(3)]
    out_t = nc.dram_tensor("out", [SHARD, NCLS], f32, kind="ExternalOutput")

    with tile.TileContext(nc) as tc:
        nc.gpsimd.load_library(library_config.mlp)
        with (
            tc.tile_pool(name="dram", bufs=1, space="DRAM") as dram,
            tc.tile_pool(name="accp", bufs=NT) as accp,
            tc.tile_pool(name="singles", bufs=1) as singles,
            tc.tile_pool(name="glo", bufs=10) as glo_pool,
            tc.tile_pool(name="ghi", bufs=10) as ghi_pool,
            tc.tile_pool(name="sbat", bufs=4) as sb_pool,
            tc.tile_pool(name="work", bufs=6) as work,
            tc.tile_pool(name="psum_p", bufs=3, space="PSUM") as psum_p,
            tc.tile_pool(name="psum_z", bufs=3, space="PSUM") as psum_z,
        ):
            ag1_in = [dram.tile([NT_A * P, D], b16, name=f"ag1_in{l}",
                                tag=f"ag1_in{l}") for l in range(2)]
            ag1_out = [dram.tile([REG_A, D], b16, name=f"ag1_out{l}",
                                 tag=f"ag1_out{l}") for l in range(2)]
            ag2_in = [dram.tile([NT_B * P, D], b16, name=f"ag2_in{l}",
                                tag=f"ag2_in{l}") for l in range(2)]
            ag2_out = [dram.tile([REG_B, D], b16, name=f"ag2_out{l}",
                                 tag=f"ag2_out{l}") for l in range(2)]

            idx_lo_t = singles.tile([P, nbl * BATCH // 16], mybir.dt.int16,
                                    tag="idxlo")
            idx_hi_t = singles.tile([P, nbh * BATCH // 16], mybir.dt.int16,
                                    tag="idxhi")
            nc.sync.dma_start(out=idx_lo_t[:], in_=ilo[:])
            nc.sync.dma_start(out=idx_hi_t[:], in_=ihi[:])
            w_t, b_t = [], []
            for i in range(3):
                nout = D if i < 2 else NCLS
                wt = singles.tile([D, nout], b16, name=f"w{i}", tag=f"w{i}")
                bt = singles.tile([1, nout], b16, name=f"b{i}", tag=f"b{i}")
                nc.sync.dma_start(out=wt[:], in_=w_in[i][:])
                nc.sync.dma_start(out=bt[:], in_=b_in[i][:])
                w_t.append(wt)
                b_t.append(bt)
            ones_t = singles.tile([1, P], b16, tag="ones")
            nc.vector.memset(ones_t[:], 1.0)

            qrr = [0]

            def issue_gather(table_ap, idx_tile, b, pool):
                g = pool.tile([P, BATCH_CH, D], b16)
                nc.gpsimd.dma_gather(
                    g[:], table_ap,
                    idx_tile[:, (b * BATCH // 16):((b + 1) * BATCH // 16)],
                    BATCH, BATCH, D,
                    single_packet=False, queue_num=qrr[0] % NQ,
                )
                qrr[0] += 1
                return g

            def table(l):
                if l == 0:
                    return xbf[0:REG_A, :], xbf[REG_A:PAD_N, :]
                return ag1_out[(l - 1) % 2][:], ag2_out[(l - 1) % 2][:]

            g_lo = [issue_gather(table(0)[0], idx_lo_t, b, glo_pool)
                    for b in range(nbl)]
            g_hi = [issue_gather(table(0)[1], idx_hi_t, b, ghi_pool)
                    for b in range(nbh)]

            for l in range(3):
                nout = D if l < 2 else NCLS
                s_bufs = []
                for b in range(nsb):
                    c0 = b * BATCH_CH
                    c1 = min(nchunk, c0 + BATCH_CH)
                    sb = sb_pool.tile([P, BATCH_CH, P], b16)
                    nc.sync.dma_start(out=sb[:, 0:(c1 - c0), :],
                                      in_=s_in[:, c0:c1, :])
                    s_bufs.append(sb)

                # ---- phase LO: region-A chunks -> acc (one tile per t)
                accs = []
                for t in range(NT):
                    nch = int(nl[t])
                    at = accp.tile([P, P], f32, tag="acc")
                    accs.append(at)
                    if nch == 0:
                        nc.vector.memset(at[:], 0.0)
                        continue
                    pp = psum_p.tile([P, P], f32, tag="pp")
                    for i in range(nch):
                        j = int(lo_off[t]) + i
                        nc.tensor.matmul(
                            out=pp[:],
                            lhsT=g_lo[j // BATCH_CH][:, j % BATCH_CH, :],
                            rhs=s_bufs[j // BATCH_CH][:, j % BATCH_CH, :],
                            start=(i == 0), stop=(i == nch - 1),
                        )
                    nc.vector.tensor_copy(out=at[:], in_=pp[:])

                # ---- phase HI: region-B chunks + Linear + norm
                g_lo_next, g_hi_next = [], []
                for t in range(NT):
                    nch = int(nh[t])
                    propT = work.tile([P, P], b16, tag="propT")
                    if nch > 0:
                        pp = psum_p.tile([P, P], f32, tag="pp")
                        for i in range(nch):
                            j = int(hi_off[t]) + i
                            k = llo_ch + j
                            nc.tensor.matmul(
                                out=pp[:],
                                lhsT=g_hi[j // BATCH_CH][:, j % BATCH_CH, :],
                                rhs=s_bufs[k // BATCH_CH][:, k % BATCH_CH, :],
                                start=(i == 0), stop=(i == nch - 1),
                            )
                        nc.vector.tensor_add(out=propT[:], in0=accs[t][:],
                                             in1=pp[:])
                    else:
                        nc.vector.tensor_copy(out=propT[:], in_=accs[t][:])
                    pz = psum_z.tile([P, nout], f32, tag="pz")
                    nc.tensor.matmul(out=pz[:], lhsT=propT[:], rhs=w_t[l][:],
                                     start=True, stop=False)
                    nc.tensor.matmul(out=pz[:], lhsT=ones_t[:], rhs=b_t[l][:],
                                     start=False, stop=True)
                    if l < 2:
                        ht = work.tile([P, D], f32, tag="ht")
                        nc.scalar.activation(
                            out=ht[:], in_=pz[:],
                            func=mybir.ActivationFunctionType.Relu)
                        sq = work.tile([P, D], f32, tag="sq")
                        ss = work.tile([P, 1], f32, tag="ss")
                        nc.scalar.activation(
                            out=sq[:], in_=ht[:],
                            func=mybir.ActivationFunctionType.Square,
                            accum_out=ss[:])
                        nc.scalar.activation(
                            out=ss[:], in_=ss[:],
                            func=mybir.ActivationFunctionType.Sqrt)
                        nc.vector.tensor_scalar_max(out=ss[:], in0=ss[:],
                                                    scalar1=float(EPS))
                        nc.vector.reciprocal(out=ss[:], in_=ss[:])
                        hb = work.tile([P, D], b16, tag="hb")
                        nc.scalar.activation(
                            out=hb[:], in_=ht[:],
                            func=mybir.ActivationFunctionType.Copy,
                            scale=ss[:])
                        if t < NT_A:
                            nc.sync.dma_start(
                                out=ag1_in[l][t * P:(t + 1) * P, :], in_=hb[:])
                        else:
                            tb = t - NT_A
                            nc.sync.dma_start(
                                out=ag2_in[l][tb * P:(tb + 1) * P, :],
                                in_=hb[:])
                    else:
                        zt = work.tile([P, NCLS], f32, tag="zt")
                        nc.vector.tensor_copy(out=zt[:], in_=pz[:])
                        nc.sync.dma_start(
                            out=out_t[t * P:(t + 1) * P, :], in_=zt[:])

                    if l < 2 and t == NT_A - 1:
                        nc.gpsimd.collective_compute(
                            "AllGather", mybir.AluOpType.bypass,
                            ins=[ag1_in[l].opt()], outs=[ag1_out[l].opt()],
                            replica_groups=[list(range(NCORES))],
                        )
                        g_lo_next = [issue_gather(table(l + 1)[0], idx_lo_t,
                                                  b, glo_pool)
                                     for b in range(nbl)]
                if l < 2:
                    nc.gpsimd.collective_compute(
                        "AllGather", mybir.AluOpType.bypass,
                        ins=[ag2_in[l].opt()], outs=[ag2_out[l].opt()],
                        replica_groups=[list(range(NCORES))],
                    )
                    g_hi_next = [issue_gather(table(l + 1)[1], idx_hi_t,
                                              b, ghi_pool)
                                 for b in range(nbh)]
                g_lo, g_hi = g_lo_next, g_hi_next
    nc.compile()
    return nc


_CACHE = {}


def _get_program(sched):
    key = (sched["nchunk"], sched["nbl"], sched["nbh"],
           tuple(sched["nl"]), tuple(sched["nh"]))
    if key not in _CACHE:
        _CACHE[key] = _build(sched)
    return _CACHE[key]


# ---------------------------------------------------------------- entry
def kernel(x, edge_index, C_vals, W1, b1, W2, b2, W3, b3):
    from concourse.bass_utils import run_bass_kernel_spmd

    x = np.asarray(x)
    sched = _prepare(edge_index, C_vals)
    nc = _get_program(sched)

    # x rows permuted into region layout (region A first, then B)
    nodes = np.arange(PAD_N)
    hh, pos = _region_pos(nodes)
    gpos = np.where(hh == 0, pos, REG_A + pos)
    xp = np.zeros((PAD_N, D), np.float32)
    xp[:N] = x
    xbf = np.zeros((PAD_N, D), bf16)
    xbf[gpos] = xp.astype(bf16)

    common = {
        "xbf": xbf,
        "W1": np.asarray(W1).astype(bf16),
        "b1": np.asarray(b1).astype(bf16).reshape(1, D),
        "W2": np.asarray(W2).astype(bf16),
        "b2": np.asarray(b2).astype(bf16).reshape(1, D),
        "W3": np.asarray(W3).astype(bf16),
        "b3": np.asarray(b3).astype(bf16).reshape(1, NCLS),
    }
    in_maps = []
    for c in range(NCORES):
        m = dict(common)
        m["s_mat"] = sched["s_mat"][c]
        m["idx_lo"] = sched["idx_lo"][c]
        m["idx_hi"] = sched["idx_hi"][c]
        in_maps.append(m)

    trace = bool(int(os.environ.get("GNN_TRACE", "0")))
    kwargs = {}
    if trace:
        import trace_utils
        trace_utils.install()
        kwargs = dict(trace=True, tmpdir="/tmp/gnn_trace")

    res = run_bass_kernel_spmd(nc, in_maps, core_ids=list(range(NCORES)),
                               **kwargs)
    if trace and res.exec_time_ns is not None:
        print(f"HW exec time: {res.exec_time_ns} ns")

    out = np.concatenate([res.results[c]["out"] for c in range(NCORES)], axis=0)
    return np.ascontiguousarray(out[:N])
